# revision 18
# baseline (speedup 1.0000x reference)
"""GroupedQueryAttention TRN2 Bass kernel, 8-way (kv-group, batch) sharded.

B=2, S=2048, E=2048, H=16 q-heads, KVH=4 kv-heads, HD=128.
Core d = (g, b) with g = d//2 (kv group), b = d%2 (batch): it computes the
4 q-heads of group g and kv-head g for batch b only.  Zero redundant
projection work; every core touches only half of x (its batch) and emits a
[S, E] partial (contraction over its 512 head dims); the host sums 4
partials per batch.

Precision: fp32r datapath on the PE; fp32 PSUM; fp16 for the attention
probabilities (exp is biased by -5 so fp16 can't overflow; the 2^-12
relative rounding of P is far inside the error budget), the causal masks,
the cos/sin tables (table quantization only -- the rope multiplies run in
fp32 on the DVE), and the output partials.

DMA: all tensors are host-pre-arranged so every transfer is contiguous per
partition (128 descriptors of 8..32 KiB) -- descriptor generation cost on
the issuing engines is negligible and the stream runs at HBM line rate.
The output is written in SBUF layout [128, tch, e] and re-ordered on host.

Layout (all matmuls natural, every fp32r matmul keeps >=256 moving rows to
stay at the 1 cycle/row full rate):
  warmup: ~20 tiny f16 matmuls on a memset tile so the PE HAM clock gate
    is at 8/8 (2.4 GHz) before the first real matmul.
  phase 1, per 512-token tile, two passes to fit PSUM (A: q0,q1,k / B:
    q2,q3,v): qT/kT/vT[hd, tok] = W.T @ xT with x streamed through a
    2-deep tile pool, RoPE on the PSUM->SBUF epilogue (1/sqrt(HD) folded
    into wq), v transposed to natural [tok, hd] via the PE.
  attention per (q-tile, head), flash-style over PAIRS of 128-wide key
    chunks: scoresT[kt, qt] = kT_chunk.T @ qT_tile -> exp(.-5) -> fp16 P;
    ctxT[hd, qt] += v_chunk.T @ P.  Causal: only chunks up to the
    diagonal; all consumers slice from dl2 = min(delta, 256) so the junk
    below the mask never feeds anything that is read, and diagonal exps
    are range-sliced per half (no stale-PSUM reads, no zeroing pass).
    The triangular mask multiply covers only the 128-wide diagonal band
    (plus the [256:384) pad of the last chunk).  sumexp via ones-matmuls
    (pairs pre-added on the DVE).  All consumers of a pair's P are
    emitted DEPTH pairs late so the PE never queues a matmul behind a
    fresh Act/DVE dep.
  out_proj per 128-token chunk of the PREVIOUS q-tile (emitted after head
    0 of the next q-tile normalizes): out[tok, e] = sum_h ctx_h.T @ wo_h,
    PSUM drained by the DVE into a [128, E] fp16 staging tile, one DMA
    per token chunk.
"""
import sys
sys.path.insert(0, '/opt/trn_rl_repo')

import numpy as np
from contextlib import ExitStack

import concourse.bass as bass
import concourse.bacc as bacc
import concourse.tile as tile
from concourse import mybir
from concourse.bass_utils import run_bass_kernel_spmd
from concourse.alu_op_type import AluOpType

F32 = mybir.dt.float32
F32R = mybir.dt.float32r
F16 = mybir.dt.float16
EXP = mybir.ActivationFunctionType.Exp

B, S, E = 2, 2048, 2048
H, KVH, HD = 16, 4, 128
NCORES = 8
NT = 512                   # token tile (matmul free dim)
NTT = S // NT              # 4 token tiles per core
KC = E // 128              # 16 contraction chunks for projections
KB = S // 128              # 16 key chunks per core
NQH = 4                    # q heads per core
ROPE_BASE = 10000.0
EXP_BIAS = -5.0            # exp(s-5): keeps fp16 P finite for scores < 16
NWARM = 28

_CACHE = {}


def _emit(nc, tc, ctx):
    x4_d = nc.declare_dram_parameter("x4", [NTT, 128, KC, NT], F32R, isOutput=False)
    wq_d = nc.declare_dram_parameter("wq4", [NQH, 128, KC, HD], F32R, isOutput=False)
    wk_d = nc.declare_dram_parameter("wk", [128, KC, HD], F32R, isOutput=False)
    wv_d = nc.declare_dram_parameter("wv", [128, KC, HD], F32R, isOutput=False)
    wo_d = nc.declare_dram_parameter("wo", [128, NQH, E], F32R, isOutput=False)
    cos_d = nc.declare_dram_parameter("cos", [HD, S], F16, isOutput=False)
    sinm_d = nc.declare_dram_parameter("sinm", [HD, S], F16, isOutput=False)
    masks_d = nc.declare_dram_parameter("masks", [128, 4, NT], F16, isOutput=False)
    ident_d = nc.declare_dram_parameter("ident", [128, 128], F32R, isOutput=False)
    onec_d = nc.declare_dram_parameter("onec", [128, 128], F32R, isOutput=False)
    out_d = nc.declare_dram_parameter("out", [128, KB, E], F16, isOutput=True)

    persist = ctx.enter_context(tc.tile_pool(name="persist", bufs=1))
    qT = [persist.tile([HD, S], F32R, name=f"qT{i}") for i in range(NQH)]
    kT = persist.tile([HD, S], F32R)
    v_sb = persist.tile([128, KB, HD], F32R)    # v natural: [tok%128, blk, hd]
    wq_s = persist.tile([128, NQH, KC, HD], F32R)  # [p, head, k, hd]
    wk_s = persist.tile([128, KC, HD], F32R)
    wv_s = persist.tile([128, KC, HD], F32R)
    cos_s = persist.tile([HD, S], F16)
    sinm_s = persist.tile([HD, S], F16)
    masks_s = persist.tile([128, 4, NT], F16)
    ident = persist.tile([128, 128], F32R)
    ones_col = persist.tile([128, 128], F32R)
    dz = persist.tile([128, 128], F16)
    ebias = persist.tile([128, 1], F32)

    # ------------- warmup: keep the PE HAM clock gate at 8/8 --------------
    nc.vector.memset(dz[:], 0.0)
    nc.vector.memset(ebias[:], EXP_BIAS)
    with ExitStack() as wm:
        wpool = wm.enter_context(tc.tile_pool(name="warm", bufs=2, space="PSUM"))
        for _ in range(NWARM):
            wt = wpool.tile([128, 128], F32, tag="w", name="wt")
            nc.tensor.matmul(wt[:], dz[:], dz[:], start=True, stop=True)

    # ---------------- phase 1: projections + RoPE + v transpose ----------------
    with ExitStack() as p1:
        xqpool = p1.enter_context(tc.tile_pool(name="xqpool", bufs=16))
        rope = p1.enter_context(tc.tile_pool(name="rope", bufs=3))
        vstage = p1.enter_context(tc.tile_pool(name="vstage", bufs=1))
        psA = p1.enter_context(tc.tile_pool(name="psA", bufs=1, space="PSUM"))
        psB = p1.enter_context(tc.tile_pool(name="psB", bufs=1, space="PSUM"))
        pst = p1.enter_context(tc.tile_pool(name="pst", bufs=2, space="PSUM"))

        RR = [nc.sync, nc.scalar, nc.gpsimd]

        def load_x8(tt, engines):
            # eight 0.5 MiB eighth-tile DMAs (4 KiB contiguous per
            # partition) spread round-robin over the given queues: the PE
            # trails the stream chunk-pair by chunk-pair, so DMA waits stay
            # under the ~3.4us HAM idle window and the clock holds 2.4 GHz
            es = []
            for e in range(8):
                xt = xqpool.tile([128, 2, NT], F32R, tag="x", name="xe")
                engines[e % len(engines)].dma_start(
                    xt[:], x4_d[tt, :, 2 * e:2 * e + 2, :])
                es.append(xt)
            return es

        # phase-1 loads are emitted in PE-consumption order, round-robin
        # across sync/scalar/gpsimd: the SDMA engines serve the queues
        # fairly, so per-queue FIFO depth is what decides arrival time
        xt0, xt1 = [], []
        nc.gpsimd.dma_start(wq_s[:, 0, 0, :], wq_d[0, :, 0, :])
        for e in range(2):
            xt0.append(xqpool.tile([128, 2, NT], F32R, tag="x", name="xe"))
        nc.sync.dma_start(xt0[0][:], x4_d[0, :, 0:2, :])
        nc.scalar.dma_start(xt0[1][:], x4_d[0, :, 2:4, :])
        nc.gpsimd.dma_start(wq_s[:, 0, 1:6, :], wq_d[0, :, 1:6, :])
        nc.sync.dma_start(wq_s[:, 0, 6:11, :], wq_d[0, :, 6:11, :])
        nc.scalar.dma_start(wq_s[:, 0, 11:16, :], wq_d[0, :, 11:16, :])
        for e in range(2, 8):
            xt = xqpool.tile([128, 2, NT], F32R, tag="x", name="xe")
            RR[e % 3].dma_start(xt[:], x4_d[0, :, 2 * e:2 * e + 2, :])
            xt0.append(xt)
        nc.sync.dma_start(wq_s[:, 1, :, :], wq_d[1, :, :, :])
        nc.scalar.dma_start(wk_s[:], wk_d[:, :, :])
        nc.gpsimd.dma_start(wv_s[:], wv_d[:, :, :])
        nc.scalar.dma_start(ident[:], ident_d[:, :])
        nc.sync.dma_start(wq_s[:, 2, :, :], wq_d[2, :, :, :])
        nc.scalar.dma_start(wq_s[:, 3, :, :], wq_d[3, :, :, :])
        xt1 = []
        for e in range(8):
            xt = xqpool.tile([128, 2, NT], F32R, tag="x", name="xe")
            RR[e % 3].dma_start(xt[:], x4_d[1, :, 2 * e:2 * e + 2, :])
            xt1.append(xt)
        nc.gpsimd.dma_start(cos_s[:], cos_d[:, :])
        nc.sync.dma_start(sinm_s[:], sinm_d[:, :])
        nc.scalar.dma_start(ones_col[:], onec_d[:, :])
        nc.gpsimd.dma_start(masks_s[:], masks_d[:, :, :])

        def rope_drain(psum, dest, t0):
            # dest = psum*cos + swap_halves(psum)*sinm, computed in fp32
            # (sinm has -sin in the top half).  The PSUM-reading multiply
            # goes first so the bank frees as early as possible.
            d = dest[:, t0:t0 + NT]
            nc.vector.tensor_tensor(d, psum[:], cos_s[:, t0:t0 + NT],
                                    AluOpType.mult)
            sw = rope.tile([HD, NT], F32, tag="sw")
            nc.scalar.copy(sw[0:64, :], psum[64:128, :])
            nc.scalar.copy(sw[64:128, :], psum[0:64, :])
            nc.vector.tensor_tensor(sw[:], sw[:], sinm_s[:, t0:t0 + NT],
                                    AluOpType.mult)
            nc.vector.tensor_tensor(d, d, sw[:], AluOpType.add)

        def qpass(pool, tag, wsl, dest, t0, xq):
            ps = pool.tile([HD, NT], F32, tag=tag)
            for k in range(KC):
                nc.tensor.matmul(ps[:], wsl[:, k, :], xq[k // 2][:, k % 2, :],
                                 start=(k == 0), stop=(k == KC - 1))
            rope_drain(ps, dest, t0)

        def vpass(tt, xq):
            pv = psB.tile([HD, NT], F32, tag="v")
            for k in range(KC):
                nc.tensor.matmul(pv[:], wv_s[:, k, :], xq[k // 2][:, k % 2, :],
                                 start=(k == 0), stop=(k == KC - 1))
            vT_s = vstage.tile([HD, NT], F32R, tag="vT")
            nc.scalar.copy(vT_s[:], pv[:])
            for c in range(NT // 128):
                tp = pst.tile([128, 128], F32R, tag="tp")
                nc.tensor.matmul(tp[:], vT_s[:, c * 128:(c + 1) * 128], ident[:],
                                 is_transpose=True)
                nc.vector.tensor_copy(v_sb[:, tt * 4 + c, :], tp[:])

        # tile 0 runs only q0/q1/k/v (8.2 MiB of critical DMA instead of
        # 10.4); its q2/q3 sub-passes run early in tile 1 while tile 1's
        # x streams in -- the DMA-bound head starves the PE less
        xq_t = {0: xt0, 1: xt1}
        for tt in range(NTT):
            t0 = tt * NT
            xq = xq_t.pop(tt)
            if tt + 2 < NTT:
                xq_t[tt + 2] = load_x8(tt + 2, [nc.sync, nc.gpsimd])
            if tt == 1:
                # tile0's deferred q2/q3 run FIRST: their data (x0 resident,
                # wq heads 2/3) lands before x1 finishes streaming, so they
                # fill the x1 wait instead of queueing behind it
                qpass(psB, "q2", wq_s[:, 2], qT[2], 0, xq0_save)
                qpass(psB, "q3", wq_s[:, 3], qT[3], 0, xq0_save)
            qpass(psA, "q0", wq_s[:, 0], qT[0], t0, xq)
            qpass(psA, "q1", wq_s[:, 1], qT[1], t0, xq)
            qpass(psA, "k", wk_s, kT, t0, xq)
            if tt == 0:
                xq0_save = xq
            else:
                qpass(psB, "q2", wq_s[:, 2], qT[2], t0, xq)
                qpass(psB, "q3", wq_s[:, 3], qT[3], t0, xq)
            vpass(tt, xq)

    # ---------- phase 2: attention + out_proj, interleaved per q-tile ----------
    with ExitStack() as p2:
        wopool = p2.enter_context(tc.tile_pool(name="wopool", bufs=1))
        ppool = p2.enter_context(tc.tile_pool(name="ppool", bufs=6))
        paddp = p2.enter_context(tc.tile_pool(name="paddp", bufs=4))
        bcsp = p2.enter_context(tc.tile_pool(name="bcsp", bufs=2))
        cxp = p2.enter_context(tc.tile_pool(name="cxp", bufs=2))
        obp = p2.enter_context(tc.tile_pool(name="obp", bufs=2))
        # PSUM budget (8 banks): scores pairs 2x2 | ctx 1 | sumexp 1 | out 2
        pss = p2.enter_context(tc.tile_pool(name="pss", bufs=2, space="PSUM"))
        psc = p2.enter_context(tc.tile_pool(name="psc", bufs=1, space="PSUM"))
        psn = p2.enter_context(tc.tile_pool(name="psn", bufs=1, space="PSUM"))
        pso = p2.enter_context(tc.tile_pool(name="pso", bufs=2, space="PSUM"))

        # issued from sync (nearly idle): the transfer starts as soon as the
        # arena under this tile frees at the end of phase 1, ~25us before
        # out_proj(qt0) needs it.  The scalar queue would serialize it
        # behind all of phase 1's rope copies.
        wo_s = wopool.tile([128, NQH, E], F32R)  # [hd%128, head, e]
        nc.sync.dma_start(wo_s[:], wo_d[:, :, :])

        def out_proj_steps(qt, cx):
            # generator: one (token-chunk, e-chunk) block per step, so the
            # blocks can be zipped between attention pairs; each token
            # chunk accumulates its full-E row in SBUF then goes out in a
            # single contiguous DMA
            last = qt == NTT - 1
            for tc4 in range(NT // 128):
                tch = qt * (NT // 128) + tc4
                ob = obp.tile([128, E], F16, tag="ob")
                for ech in range(E // NT):
                    esl = slice(ech * NT, (ech + 1) * NT)
                    op = pso.tile([128, NT], F32, tag="o")
                    for h in range(NQH):
                        nc.tensor.matmul(op[:], cx[h][:, tc4 * 128:(tc4 + 1) * 128],
                                         wo_s[:, h, esl],
                                         start=(h == 0), stop=(h == NQH - 1))
                    if last and tc4 >= 2:
                        # tail: the Act engine is free once the exps dry up,
                        # and per-chunk DMAs start draining immediately
                        (nc.vector.tensor_copy if ech % 2 == 0
                         else nc.scalar.copy)(ob[:, esl], op[:])
                        if tc4 == 3:
                            (nc.sync if ech % 2 == 0 else nc.gpsimd).dma_start(
                                out_d[:, tch, esl], ob[:, esl])
                    else:
                        nc.vector.tensor_copy(ob[:, esl], op[:])
                    yield
                if not (last and tc4 == 3):
                    (nc.sync if tc4 % 2 == 0 else nc.gpsimd).dma_start(
                        out_d[:, tch, :], ob[:])

        DEPTH = 3  # deferral depth in pairs: PE always has ~3us queued
        pending = None
        for qt in range(NTT):
            npairs = 2 * (qt + 1)
            nk = 4 * (qt + 1)
            q_sl = slice(qt * NT, (qt + 1) * NT)

            def delta(kc):
                # fully-masked column prefix of a diagonal chunk, clamped so
                # every fp32r matmul keeps >=256 moving rows (full rate);
                # the [256:384) pad of the last chunk is zeroed by the mask
                d = (kc - 4 * qt) * 128 if kc >= 4 * qt else 0
                return min(d, NT - 256) if d else 0

            cx = [cxp.tile([HD, NT], F32R, tag=f"cx{i}", name=f"cx{i}")
                  for i in range(NQH)]
            # per-head PSUM state, created lazily at first deferred flush
            hstate = {}

            def flush(item):
                # emit the ctx/sump matmuls for a pair DEPTH pairs after its
                # scores/exp were issued, so the PE never reaches a matmul
                # whose Act/DVE producer hasn't finished
                h, j, diag, pexp, padd = item
                if h not in hstate:
                    ctxp = psc.tile([HD, NT], F32, tag="ctx", name="ctxp")
                    sump = psn.tile([128, NT], F32, tag="sum", name="sump")
                    hstate[h] = (ctxp, sump)
                ctxp, sump = hstate[h]
                for half in (0, 1):
                    kc = 2 * j + half
                    dl = delta(kc)
                    st, sp_ = (kc == 0), (kc == nk - 1)
                    nc.tensor.matmul(
                        ctxp[:, dl:], v_sb[:, kc, :],
                        pexp[:, half, dl:], start=st, stop=sp_)
                    if diag:
                        nc.tensor.matmul(sump[:, dl:], ones_col[:],
                                         pexp[:, half, dl:],
                                         start=(qt == 0 and kc == 0),
                                         stop=sp_)
                if not diag:
                    nc.tensor.matmul(sump[:], ones_col[:], padd[:],
                                     start=(j == 0), stop=False)
                if j == npairs - 1:
                    # normalize straight out of PSUM: recip of the broadcast
                    # sumexp (all-ones stationary), one fused multiply
                    bcs = bcsp.tile([128, NT], F32, tag="bcs")
                    nc.vector.reciprocal_approx_fast(bcs[:], sump[:])
                    nc.vector.tensor_tensor(cx[h][:], ctxp[:], bcs[:],
                                            AluOpType.mult)
                    return h
                return None

            dq = []
            for h in range(NQH):
                for j in range(npairs):
                    diag = j >= 2 * qt
                    sp2 = pss.tile([128, 2, NT], F32, tag="s")
                    for half in (0, 1):
                        kc = 2 * j + half
                        dl = delta(kc)
                        nc.tensor.matmul(
                            sp2[:, half, dl:],
                            kT[:, kc * 128:(kc + 1) * 128],
                            qT[h][:, q_sl][:, dl:])
                    pexp = ppool.tile([128, 2, NT], F32R, tag="p")
                    padd = None
                    if diag:
                        # range-sliced exp per half: nothing below dl is ever
                        # written or read, so no stale-PSUM zeroing is needed
                        for half in (0, 1):
                            kc = 2 * j + half
                            dl = delta(kc)
                            nc.scalar.activation(pexp[:, half, dl:],
                                                 sp2[:, half, dl:], EXP,
                                                 bias=ebias[:])
                        for half in (0, 1):
                            kc = 2 * j + half
                            m = kc - 4 * qt
                            dl = delta(kc)
                            hi = NT if m == 3 else (m + 1) * 128
                            nc.vector.tensor_tensor(
                                pexp[:, half, dl:hi], pexp[:, half, dl:hi],
                                masks_s[:, m, dl:hi], AluOpType.mult)
                    else:
                        nc.scalar.activation(pexp[:], sp2[:], EXP,
                                             bias=ebias[:])
                        padd = paddp.tile([128, NT], F32R, tag="padd")
                        nc.vector.tensor_tensor(padd[:], pexp[:, 0, :],
                                                pexp[:, 1, :], AluOpType.add)
                    dq.append((h, j, diag, pexp, padd))
                    if len(dq) > DEPTH:
                        done_h = flush(dq.pop(0))
                        if done_h == 0 and pending is not None:
                            # head 0 normalized: the previous q-tile's
                            # out_proj runs here, while this q-tile's
                            # remaining exps stream on the Act engine
                            for _ in pending:
                                pass
                            pending = None
            while dq:
                flush(dq.pop(0))
            if pending is not None:  # qt0 spill (fewer pairs than DEPTH)
                for _ in pending:
                    pass
            pending = out_proj_steps(qt, cx)
        for _ in pending:
            pass


def _build():
    if "nc" in _CACHE:
        return _CACHE["nc"]
    nc = bacc.Bacc("TRN2", target_bir_lowering=False, debug=False,
                   num_devices=NCORES)
    with tile.TileContext(nc) as tc:
        with nc.allow_low_precision(reason="float32r operands for full-rate PE"):
            with ExitStack() as ctx:
                _emit(nc, tc, ctx)
    nc.compile()
    _CACHE["nc"] = nc
    return nc


def _host_consts():
    if "consts" in _CACHE:
        return _CACHE["consts"]
    # RoPE tables, computed in float32 like the reference
    inv_freq = (1.0 / (ROPE_BASE ** (np.arange(0, HD, 2, dtype=np.float32) / HD))
                ).astype(np.float32)
    t = np.arange(S, dtype=np.float32)
    freqs = np.outer(t, inv_freq).astype(np.float32)          # [S, 64]
    emb = np.concatenate([freqs, freqs], axis=-1)             # [S, HD]
    cos_t = np.ascontiguousarray(np.cos(emb).T.astype(np.float16))  # [HD, S]
    sin_t = np.sin(emb).T.astype(np.float32)
    sinm_t = np.ascontiguousarray(
        np.concatenate([-sin_t[:64], sin_t[64:]], axis=0)).astype(np.float16)
    # causal masks for the 4 diagonal 128-chunk offsets within a 512 q-tile,
    # pre-transposed to [p, m, j]
    p = np.arange(128)[:, None]
    j = np.arange(NT)[None, :]
    masks = np.stack([(m * 128 + p <= j) for m in range(4)]).astype(np.float16)
    masks = np.ascontiguousarray(masks.transpose(1, 0, 2))
    ident = np.eye(128, dtype=np.float32)
    onec = np.ones((128, 128), np.float32)
    _CACHE["consts"] = (cos_t, sinm_t, masks, ident, onec)
    return _CACHE["consts"]


def make_in_maps(x, wq, wk, wv, wo):
    cos_t, sinm_t, masks, ident, onec = _host_consts()
    x = np.asarray(x, dtype=np.float32)
    scale = np.float32(1.0 / np.sqrt(HD))
    wq32 = np.asarray(wq, dtype=np.float32) * scale
    wk32 = np.asarray(wk, dtype=np.float32)
    wv32 = np.asarray(wv, dtype=np.float32)
    wo32 = np.asarray(wo, dtype=np.float32)
    # x per batch, pre-arranged to [tile, p, k, t] so each DMA is contiguous
    # per partition: element (tt, p, k, u) = x[b].T[k*128+p, tt*512+u]
    x4 = [np.ascontiguousarray(
        x[b].T.reshape(KC, 128, NTT, NT).transpose(2, 1, 0, 3))
        for b in range(B)]
    in_maps = []
    for d in range(NCORES):
        g, b = d // 2, d % 2
        wqg = wq32[:, g * NQH * HD:(g + 1) * NQH * HD]   # [E, 512]
        wq4 = np.ascontiguousarray(
            wqg.reshape(KC, 128, NQH, HD).transpose(2, 1, 0, 3))
        wkg = np.ascontiguousarray(
            wk32[:, g * HD:(g + 1) * HD].reshape(KC, 128, HD).transpose(1, 0, 2))
        wvg = np.ascontiguousarray(
            wv32[:, g * HD:(g + 1) * HD].reshape(KC, 128, HD).transpose(1, 0, 2))
        wog = np.ascontiguousarray(
            wo32[g * NQH * HD:(g + 1) * NQH * HD, :]
            .reshape(NQH, 128, E).transpose(1, 0, 2))
        in_maps.append({
            "x4": x4[b],
            "wq4": wq4, "wk": wkg, "wv": wvg, "wo": wog,
            "cos": cos_t, "sinm": sinm_t, "masks": masks, "ident": ident,
            "onec": onec,
        })
    return in_maps


def kernel(x, wq, wk, wv, wo, attn_mask):
    nc = _build()
    in_maps = make_in_maps(x, wq, wk, wv, wo)
    res = run_bass_kernel_spmd(nc, in_maps, list(range(NCORES)))
    out = np.empty((B, S, E), np.float32)
    for b in range(B):
        o = res.results[b]["out"].astype(np.float64)
        for g in range(1, KVH):
            o += res.results[2 * g + b]["out"]
        # [128, tch, e] -> [tok, e]
        out[b] = o.transpose(1, 0, 2).reshape(S, E).astype(np.float32)
    return out
